# revision 1
# baseline (speedup 1.0000x reference)
"""Trainium2 Bass kernel for nn_Decoder_24764781429449 (GNN message passing).

Math (per layer l of 3, N=4096 nodes, K=48 neighbors, C=128 channels, H=512):
    base   = concat([node0, zeros, edge])                  # fixed context
    mlp_in = concat([x, base])                             # [N,K,512]
    h1  = gelu(mlp_in @ W1 + b1)
    h2  = gelu(h1 @ W2 + b2)
    msg = h2 @ W3 + b3
    x   = LN1(x + sum_k(msg)/30);  x = LN2(x + dense_mlp(x));  x *= mask

Key algebraic reductions used here:
  * W1 rows 256:384 multiply zeros -> dead.
  * The x/node0 parts of the concat are shared across all K neighbors:
    h1 = gelu(edge @ W1d + pernode),  pernode = x@W1a + node0@W1b + b1.
  * sum_k (h2 @ W3 + b3) = (sum_k h2) @ W3 + K*b3; the k-sum is done by
    PSUM accumulation of per-k W3 matmuls.

Distribution: data-parallel over nodes, 512 nodes per core across 8 cores.
Edge features stay SBUF-resident (12.6MB/core), read from HBM exactly once.
All big matmuls run in float32r (TF32-like, full PE rate).
LayerNorm rsqrt is computed on the Vector engine with a bit-hack seed +
Newton iterations, so the Scalar engine runs Gelu only (no table switches).

Layout on device (per core):
  edge_km [128c, 24576]: col = half*12288 + k*256 + n  (half in {0,1}, n in 0..255)
  x / pernode: channel-major [128c, 512n]
  LayerNorm runs row-major via PE transposes: tiles [128n, 128c].
"""
import os
import numpy as np
from contextlib import ExitStack

import concourse.bass as bass
import concourse.bacc as bacc
import concourse.tile as tile
from concourse import mybir
from concourse.bass_utils import run_bass_kernel_spmd

F32 = mybir.dt.float32
F32R = mybir.dt.float32r
I32 = mybir.dt.int32
AF = mybir.ActivationFunctionType
OP = mybir.AluOpType

N, K, C, E, H, L = 4096, 48, 128, 128, 512, 3
NCORES = 8
NLOC = N // NCORES          # 512 nodes per core
NHALF = NLOC // 2           # 256
KPQ = 4                     # k-values per span
SPAN = KPQ * NHALF          # 1024 columns per span
NSPAN = K // KPQ            # 12 spans per half
SCALE = 30.0
EPS = 1e-5
MAGIC = 0x5F3759DF
NRM_T = NLOC // 128         # 4 row-major tiles of 128 nodes

_CACHED = {}
DBG = True


def _build():
    V_LAYERS = int(os.environ.get("KV_LAYERS", L))
    V_NOTAIL = os.environ.get("KV_NOTAIL", "0") == "1"
    V_REPS = int(os.environ.get("KV_REPS", "1"))
    V_ONECHUNK = os.environ.get("KV_ONECHUNK", "0") == "1"
    nc = bacc.Bacc()

    # ---------------- DRAM tensors ----------------
    edge_d = nc.dram_tensor("edge_km", [C, 2 * K * NHALF], F32R, kind="ExternalInput")
    x0_d = nc.dram_tensor("x0_ch", [C, NLOC], F32R, kind="ExternalInput")
    i128_d = nc.dram_tensor("i128", [C, C], F32R, kind="ExternalInput")
    w1a_d = nc.dram_tensor("w1a", [L, C, C], F32R, kind="ExternalInput")   # w1a[0] pre-folded with w1b[0]
    w1b_d = nc.dram_tensor("w1b", [L, C, C], F32R, kind="ExternalInput")
    w1d_d = nc.dram_tensor("w1d", [L, C, C], F32R, kind="ExternalInput")
    w2_d = nc.dram_tensor("w2", [L, C, C], F32R, kind="ExternalInput")
    w3e_d = nc.dram_tensor("w3e", [L, C, C], F32R, kind="ExternalInput")   # w3/SCALE
    dw1_d = nc.dram_tensor("dw1", [L, C, H], F32R, kind="ExternalInput")
    dw2_d = nc.dram_tensor("dw2", [L, C, 4 * C], F32R, kind="ExternalInput")
    # per-channel vectors: [L, C] each; db1 is [L, H] -> [L, 4, C]
    b1_d = nc.dram_tensor("b1", [L, C], F32, kind="ExternalInput")
    b2_d = nc.dram_tensor("b2", [L, C], F32, kind="ExternalInput")
    b3e_d = nc.dram_tensor("b3e", [L, C], F32, kind="ExternalInput")       # b3*K/SCALE
    db1_d = nc.dram_tensor("db1", [L, 4, C], F32, kind="ExternalInput")
    db2_d = nc.dram_tensor("db2", [L, C], F32, kind="ExternalInput")
    g1_d = nc.dram_tensor("ln1g", [L, C], F32, kind="ExternalInput")
    bb1_d = nc.dram_tensor("ln1b", [L, C], F32, kind="ExternalInput")
    g2_d = nc.dram_tensor("ln2g", [L, C], F32, kind="ExternalInput")
    bb2_d = nc.dram_tensor("ln2b", [L, C], F32, kind="ExternalInput")
    mask_d = nc.dram_tensor("mask_rm", [C, NRM_T], F32, kind="ExternalInput")
    out_d = nc.dram_tensor("out", [NLOC, C], F32, kind="ExternalOutput")
    if DBG:
        dbg_d = {
            "dbg_pern0": nc.dram_tensor("dbg_pern0", [C, NLOC], F32, kind="ExternalOutput"),
            "dbg_h1": nc.dram_tensor("dbg_h1", [C, SPAN], F32, kind="ExternalOutput"),
            "dbg_h2": nc.dram_tensor("dbg_h2", [C, SPAN], F32, kind="ExternalOutput"),
            "dbg_x1": nc.dram_tensor("dbg_x1", [C, NHALF], F32, kind="ExternalOutput"),
            "dbg_mv": nc.dram_tensor("dbg_mv", [C, 4], F32, kind="ExternalOutput"),
            "dbg_isd": nc.dram_tensor("dbg_isd", [C, 2], F32, kind="ExternalOutput"),
            "dbg_xln1": nc.dram_tensor("dbg_xln1", [C, NHALF], F32, kind="ExternalOutput"),
            "dbg_dh": nc.dram_tensor("dbg_dh", [C, 4 * NHALF], F32, kind="ExternalOutput"),
            "dbg_x2": nc.dram_tensor("dbg_x2", [C, NHALF], F32, kind="ExternalOutput"),
            "dbg_x3rm": nc.dram_tensor("dbg_x3rm", [C, 2 * C], F32, kind="ExternalOutput"),
            "dbg_xs1": nc.dram_tensor("dbg_xs1", [C, NLOC], F32, kind="ExternalOutput"),
        }

    def bcast_row(dram_ap):
        """Partition-broadcast a [C]-vector DRAM AP to [128, C]."""
        return bass.AP(tensor=dram_ap.tensor, offset=dram_ap.offset,
                       ap=[[0, 128]] + list(dram_ap.ap))

    with tile.TileContext(nc) as tc, ExitStack() as ctx:
        const = ctx.enter_context(tc.tile_pool(name="const", bufs=1))
        h1p = ctx.enter_context(tc.tile_pool(name="h1p", bufs=2))
        h2p = ctx.enter_context(tc.tile_pool(name="h2p", bufs=2))
        tl = ctx.enter_context(tc.tile_pool(name="tl", bufs=2))
        spanps = ctx.enter_context(tc.tile_pool(name="spanps", bufs=3, space="PSUM"))
        msump = ctx.enter_context(tc.tile_pool(name="msump", bufs=2, space="PSUM"))
        tailps = msump

        # ---------------- persistent SBUF ----------------
        edge = const.tile([C, 2 * K * NHALF], F32R)
        x0 = const.tile([C, NLOC], F32R)
        i128 = const.tile([C, C], F32R)
        w1a = const.tile([C, L * C], F32R)
        w1b = const.tile([C, L * C], F32R)
        w1d = const.tile([C, L * C], F32R)
        w2 = const.tile([C, L * C], F32R)
        w3e = const.tile([C, L * C], F32R)
        dw1 = const.tile([C, L * H], F32R)
        dw2 = const.tile([C, L * 4 * C], F32R)
        b1c = const.tile([C, L], F32)
        b2c = const.tile([C, L], F32)
        b3ec = const.tile([C, L], F32)
        db1c = const.tile([C, L * 4], F32)
        db2c = const.tile([C, L], F32)
        gbc1 = const.tile([C, L * C], F32)   # row-major per-channel broadcast tiles
        bbc1 = const.tile([C, L * C], F32)
        gbc2 = const.tile([C, L * C], F32)
        bbc2 = const.tile([C, L * C], F32)
        maskc = const.tile([C, NRM_T], F32)
        magic = const.tile([C, 1], I32)
        n0pern = const.tile([C, 2 * NLOC], F32)      # l=1,2
        pern = [const.tile([C, NLOC], F32R, name=f"pern{l}", tag=f"pern{l}") for l in range(L)]
        xs = [x0] + [const.tile([C, NLOC], F32R, name=f"x{l}", tag=f"x{l}") for l in (1, 2)]

        nc.vector.memset(magic, MAGIC)

        # ---------------- input DMAs ----------------
        for l in range(L):
            nc.sync.dma_start(w1a[:, l * C:(l + 1) * C], w1a_d.ap()[l])
            nc.sync.dma_start(w1b[:, l * C:(l + 1) * C], w1b_d.ap()[l])
            nc.sync.dma_start(w1d[:, l * C:(l + 1) * C], w1d_d.ap()[l])
            nc.sync.dma_start(w2[:, l * C:(l + 1) * C], w2_d.ap()[l])
            nc.sync.dma_start(w3e[:, l * C:(l + 1) * C], w3e_d.ap()[l])
            nc.sync.dma_start(dw1[:, l * H:(l + 1) * H], dw1_d.ap()[l])
            nc.sync.dma_start(dw2[:, l * 4 * C:(l + 1) * 4 * C], dw2_d.ap()[l])
            nc.sync.dma_start(b1c[:, l:l + 1], b1_d.ap()[l].rearrange("(c one) -> c one", one=1))
            nc.sync.dma_start(b2c[:, l:l + 1], b2_d.ap()[l].rearrange("(c one) -> c one", one=1))
            nc.sync.dma_start(b3ec[:, l:l + 1], b3e_d.ap()[l].rearrange("(c one) -> c one", one=1))
            nc.sync.dma_start(db2c[:, l:l + 1], db2_d.ap()[l].rearrange("(c one) -> c one", one=1))
            for hh in range(4):
                nc.sync.dma_start(db1c[:, l * 4 + hh:l * 4 + hh + 1],
                                  db1_d.ap()[l, hh].rearrange("(c one) -> c one", one=1))
            nc.sync.dma_start(gbc1[:, l * C:(l + 1) * C], bcast_row(g1_d.ap()[l]))
            nc.sync.dma_start(bbc1[:, l * C:(l + 1) * C], bcast_row(bb1_d.ap()[l]))
            nc.sync.dma_start(gbc2[:, l * C:(l + 1) * C], bcast_row(g2_d.ap()[l]))
            nc.sync.dma_start(bbc2[:, l * C:(l + 1) * C], bcast_row(bb2_d.ap()[l]))
        nc.sync.dma_start(i128, i128_d.ap())
        nc.sync.dma_start(maskc, mask_d.ap())
        nc.sync.dma_start(x0, x0_d.ap())
        # edge chunks: fine-grained, spread across HW and SW DMA queues
        ECH = NHALF  # 256-col chunks
        nchunks = 1 if V_ONECHUNK else (2 * K * NHALF // ECH)
        for cchunk in range(nchunks):
            sl = slice(cchunk * ECH, (cchunk + 1) * ECH)
            eng = nc.sync if cchunk % 2 == 0 else nc.gpsimd
            eng.dma_start(edge[:, sl], edge_d.ap()[:, sl])

        # ---------------- setup: n0pern (l=1,2), pern[0] ----------------
        for li, l in enumerate((1, 2)):
            pp = tailps.tile([C, NLOC], F32, tag="ps1", name="pp")
            nc.tensor.matmul(pp, w1b[:, l * C:(l + 1) * C], x0, start=True, stop=True)
            nc.vector.tensor_copy(n0pern[:, li * NLOC:(li + 1) * NLOC], pp)
        pp = tailps.tile([C, NLOC], F32, tag="ps1", name="pp")
        nc.tensor.matmul(pp, w1a[:, 0:C], x0, start=True, stop=True)
        # pern0 = pp + b1[0]  (w1a[0] holds w1a+w1b pre-folded)
        nc.vector.tensor_scalar(pern[0], pp, b1c[:, 0:1], None, op0=OP.add)

        if DBG:
            nc.sync.dma_start(dbg_d["dbg_pern0"].ap(), pern[0].bitcast(F32))

        # quake rsqrt helper: writes 1/sqrt(v+EPS) into `dst` ([128, n] fp32)
        def quake_rsqrt(pool_tile_cols, var_ap, dst):
            n = pool_tile_cols
            veps = tl.tile([C, n], F32, tag="q_veps")
            nc.vector.tensor_scalar(veps, var_ap, EPS, None, op0=OP.add)
            ush = tl.tile([C, n], I32, tag="q_ush")
            nc.vector.tensor_scalar(ush, veps.bitcast(I32), 1, None,
                                    op0=OP.logical_shift_right)
            nc.vector.scalar_tensor_tensor(
                dst.bitcast(I32), in0=magic.broadcast_to([C, n]), scalar=0,
                in1=ush, op0=OP.bypass, op1=OP.subtract)
            t = tl.tile([C, n], F32, tag="q_t")
            for _ in range(3):
                nc.vector.tensor_mul(t, dst, dst)
                nc.vector.tensor_mul(t, t, veps)
                nc.vector.tensor_scalar(t, t, -0.5, 1.5, op0=OP.mult, op1=OP.add)
                nc.vector.tensor_mul(dst, dst, t)

        # ---------------- main: layers x halves ----------------
        for _rep in range(V_REPS):
          for l in range(V_LAYERS):
            for h in range(2):
                nsl = slice(h * NHALF, (h + 1) * NHALF)
                msum = msump.tile([C, NHALF], F32, tag="ps1", name="msum")
                # ---- message-MLP spans ----
                # Software-pipelined spans: gelu-A of span s+1 is emitted
                # before gelu-B of span s so the ACT stream never waits on
                # the W2 matmuls; msum matmuls trail one more step.
                h1s, t2s, h2s = {}, {}, {}

                def emit_mm1(s):
                    t1 = spanps.tile([C, SPAN], F32, tag="span", name="t1")
                    for q in range(KPQ):
                        rsl = slice(q * NHALF, (q + 1) * NHALF)
                        col0 = 0 if V_ONECHUNK else (h * (K * NHALF) + (s * KPQ + q) * NHALF)
                        nc.tensor.matmul(t1[:, rsl], i128, pern[l][:, nsl],
                                         start=True, stop=False)
                        nc.tensor.matmul(t1[:, rsl], w1d[:, l * C:(l + 1) * C],
                                         edge[:, col0:col0 + NHALF],
                                         start=False, stop=True)
                    return t1

                def emit_geluA(s, t1):
                    h1 = h1p.tile([C, SPAN], F32R, tag="h1", name="h1")
                    nc.scalar.activation(h1, t1, AF.Gelu)
                    if DBG and l == 0 and h == 0 and s == 0:
                        nc.sync.dma_start(dbg_d["dbg_h1"].ap(), h1.bitcast(F32))
                    h1s[s] = h1

                def emit_B(s):
                    h1 = h1s.pop(s)
                    t2 = spanps.tile([C, SPAN], F32, tag="span", name="t2")
                    for j in range(2):
                        jsl = slice(j * 512, (j + 1) * 512)
                        nc.tensor.matmul(t2[:, jsl], w2[:, l * C:(l + 1) * C],
                                         h1[:, jsl], start=True, stop=True)
                    h2 = h2p.tile([C, SPAN], F32R, tag="h2", name="h2")
                    nc.scalar.activation(h2, t2, AF.Gelu, bias=b2c[:, l:l + 1])
                    if DBG and l == 0 and h == 0 and s == 0:
                        nc.sync.dma_start(dbg_d["dbg_h2"].ap(), h2.bitcast(F32))
                    h2s[s] = h2

                def emit_msum(s):
                    h2 = h2s.pop(s)
                    for q in range(KPQ):
                        rsl = slice(q * NHALF, (q + 1) * NHALF)
                        nc.tensor.matmul(msum, w3e[:, l * C:(l + 1) * C], h2[:, rsl],
                                         start=(s == 0 and q == 0),
                                         stop=(s == NSPAN - 1 and q == KPQ - 1))

                for s in range(NSPAN):
                    t1 = emit_mm1(s)
                    emit_geluA(s, t1)
                    if s >= 1:
                        emit_B(s - 1)
                    if s >= 2:
                        emit_msum(s - 2)
                emit_B(NSPAN - 1)
                emit_msum(NSPAN - 2)
                emit_msum(NSPAN - 1)
                # ---- node tail for this half ----
                # x1 = x + msum + b3e
                x1 = tl.tile([C, NHALF], F32, tag="x1")
                nc.vector.scalar_tensor_tensor(
                    x1, in0=msum, scalar=b3ec[:, l:l + 1], in1=xs[l].bitcast(F32)[:, nsl],
                    op0=OP.add, op1=OP.add)
                if DBG and l == 0 and h == 0:
                    nc.sync.dma_start(dbg_d["dbg_x1"].ap(), x1)
                # transpose to row-major
                x1rm = tl.tile([C, 2, C], F32, tag="x1rm")
                for t in range(2):
                    tp = tailps.tile([C, C], F32, tag="ps1", name="tp")
                    nc.tensor.transpose(tp, x1[:, t * C:(t + 1) * C], i128.bitcast(F32))
                    nc.vector.tensor_copy(x1rm[:, t], tp)
                # LN1 stats
                st = tl.tile([C, 2, 6], F32, tag="st")
                mv = tl.tile([C, 2, 2], F32, tag="mv")
                for t in range(2):
                    nc.vector.bn_stats(st[:, t], x1rm[:, t])
                    nc.vector.bn_aggr(mv[:, t], st[:, t])
                isd = tl.tile([C, 2], F32, tag="isd")
                quake_rsqrt(2, mv[:, :, 1], isd)
                if DBG and l == 0 and h == 0:
                    nc.sync.dma_start(dbg_d["dbg_mv"].ap(), mv.rearrange("p a b -> p (a b)"))
                    nc.sync.dma_start(dbg_d["dbg_isd"].ap(), isd)
                # apply LN1: xln = ((x1rm - mu) * is) * g + b
                xln1rm = tl.tile([C, 2, C], F32, tag="xln1rm")
                for t in range(2):
                    nc.vector.tensor_scalar(xln1rm[:, t], x1rm[:, t],
                                            mv[:, t, 0:1], isd[:, t:t + 1],
                                            op0=OP.subtract, op1=OP.mult)
                    nc.vector.tensor_mul(xln1rm[:, t], xln1rm[:, t], gbc1[:, l * C:(l + 1) * C])
                    nc.vector.tensor_add(xln1rm[:, t], xln1rm[:, t], bbc1[:, l * C:(l + 1) * C])
                # transpose back to channel-major
                xln1 = tl.tile([C, NHALF], F32R, tag="xln1")
                for t in range(2):
                    tp = tailps.tile([C, C], F32, tag="ps1", name="tp")
                    nc.tensor.transpose(tp, xln1rm[:, t], i128.bitcast(F32))
                    nc.vector.tensor_copy(xln1[:, t * C:(t + 1) * C], tp)
                if DBG and l == 0 and h == 0:
                    nc.sync.dma_start(dbg_d["dbg_xln1"].ap(), xln1.bitcast(F32))
                # dense MLP
                dh = tl.tile([C, 4 * NHALF], F32R, tag="dh")
                for hh in range(4):
                    pd = tailps.tile([C, NHALF], F32, tag="ps1", name="pd")
                    nc.tensor.matmul(pd, dw1[:, l * H + hh * C:l * H + (hh + 1) * C],
                                     xln1, start=True, stop=True)
                    nc.scalar.activation(dh[:, hh * NHALF:(hh + 1) * NHALF], pd,
                                         AF.Gelu, bias=db1c[:, l * 4 + hh:l * 4 + hh + 1])
                if DBG and l == 0 and h == 0:
                    nc.sync.dma_start(dbg_d["dbg_dh"].ap(), dh.bitcast(F32))
                pd2 = tailps.tile([C, NHALF], F32, tag="ps1", name="pd2")
                for hh in range(4):
                    nc.tensor.matmul(pd2, dw2[:, (l * 4 + hh) * C:(l * 4 + hh + 1) * C],
                                     dh[:, hh * NHALF:(hh + 1) * NHALF],
                                     start=(hh == 0), stop=(hh == 3))
                # x2 = xln1 + dense + db2
                x2 = tl.tile([C, NHALF], F32, tag="x2")
                nc.vector.scalar_tensor_tensor(
                    x2, in0=pd2, scalar=db2c[:, l:l + 1], in1=xln1.bitcast(F32),
                    op0=OP.add, op1=OP.add)
                if DBG and l == 0 and h == 0:
                    nc.sync.dma_start(dbg_d["dbg_x2"].ap(), x2)
                # transpose to row-major
                x2rm = tl.tile([C, 2, C], F32, tag="x2rm")
                for t in range(2):
                    tp = tailps.tile([C, C], F32, tag="ps1", name="tp")
                    nc.tensor.transpose(tp, x2[:, t * C:(t + 1) * C], i128.bitcast(F32))
                    nc.vector.tensor_copy(x2rm[:, t], tp)
                # LN2
                st2 = tl.tile([C, 2, 6], F32, tag="st")
                mv2 = tl.tile([C, 2, 2], F32, tag="mv")
                for t in range(2):
                    nc.vector.bn_stats(st2[:, t], x2rm[:, t])
                    nc.vector.bn_aggr(mv2[:, t], st2[:, t])
                isd2 = tl.tile([C, 2], F32, tag="isd")
                quake_rsqrt(2, mv2[:, :, 1], isd2)
                x3rm = tl.tile([C, 2, C], F32, tag="x3rm")
                for t in range(2):
                    nc.vector.tensor_scalar(x3rm[:, t], x2rm[:, t],
                                            mv2[:, t, 0:1], isd2[:, t:t + 1],
                                            op0=OP.subtract, op1=OP.mult)
                    nc.vector.tensor_mul(x3rm[:, t], x3rm[:, t], gbc2[:, l * C:(l + 1) * C])
                    nc.vector.tensor_add(x3rm[:, t], x3rm[:, t], bbc2[:, l * C:(l + 1) * C])
                    # mask (per-node = per-partition in row-major)
                    nc.vector.tensor_scalar(x3rm[:, t], x3rm[:, t],
                                            maskc[:, 2 * h + t:2 * h + t + 1], None,
                                            op0=OP.mult)
                if DBG and l == 0 and h == 0:
                    nc.sync.dma_start(dbg_d["dbg_x3rm"].ap(), x3rm.rearrange("p a b -> p (a b)"))
                if l < L - 1:
                    # transpose back into xs[l+1] and compute pern[l+1] slice
                    for t in range(2):
                        tp = tailps.tile([C, C], F32, tag="ps1", name="tp")
                        nc.tensor.transpose(tp, x3rm[:, t], i128.bitcast(F32))
                        nc.vector.tensor_copy(
                            xs[l + 1][:, h * NHALF + t * C:
                                      h * NHALF + (t + 1) * C], tp)
                    pp = tailps.tile([C, NHALF], F32, tag="ps1", name="pp")
                    nc.tensor.matmul(pp, w1a[:, (l + 1) * C:(l + 2) * C], xs[l + 1][:, nsl],
                                     start=True, stop=True)
                    nc.vector.scalar_tensor_tensor(
                        pern[l + 1][:, nsl], in0=pp,
                        scalar=b1c[:, l + 1:l + 2], in1=n0pern[:, l * NLOC + h * NHALF:l * NLOC + (h + 1) * NHALF],
                        op0=OP.add, op1=OP.add)
                else:
                    for t in range(2):
                        nc.sync.dma_start(
                            out_d.ap()[h * NHALF + t * C:h * NHALF + (t + 1) * C, :],
                            x3rm[:, t])

        if DBG:
            nc.sync.dma_start(dbg_d["dbg_xs1"].ap(), xs[1].bitcast(F32))

    nc.compile()
    return nc


def _prep_inputs(inputs):
    """Host-side: shard over nodes, relayout, fold weight-only arithmetic."""
    nf = np.asarray(inputs["node_features"], dtype=np.float32)
    ef = np.asarray(inputs["edge_features"], dtype=np.float32)
    mask = np.asarray(inputs["mask"], dtype=np.float32)
    w1 = np.asarray(inputs["msg_w1"], dtype=np.float32)
    w2 = np.asarray(inputs["msg_w2"], dtype=np.float32)
    w3 = np.asarray(inputs["msg_w3"], dtype=np.float32)

    w1a = w1[:, 0:C, :].copy()
    w1b = w1[:, C:2 * C, :].copy()
    w1d = w1[:, 3 * C:4 * C, :].copy()
    # layer 0: x == node0, fold both contributions into w1a[0]
    w1a[0] = w1a[0] + w1b[0]
    w3e = (w3 / SCALE).copy()
    b3e = (np.asarray(inputs["msg_b3"], dtype=np.float32) * (K / SCALE)).copy()
    dw2 = np.ascontiguousarray(np.asarray(inputs["d_w2"], dtype=np.float32)
                               .reshape(L, 4, C, C).transpose(0, 2, 1, 3)
                               .reshape(L, C, 4 * C))
    db1 = np.asarray(inputs["d_b1"], dtype=np.float32).reshape(L, 4, C).copy()

    shared = {
        "i128": np.eye(C, dtype=np.float32),
        "w1a": w1a, "w1b": w1b, "w1d": w1d, "w2": w2.copy(), "w3e": w3e,
        "dw1": np.asarray(inputs["d_w1"], dtype=np.float32).copy(),
        "dw2": dw2,
        "b1": np.asarray(inputs["msg_b1"], dtype=np.float32).copy(),
        "b2": np.asarray(inputs["msg_b2"], dtype=np.float32).copy(),
        "b3e": b3e, "db1": db1,
        "db2": np.asarray(inputs["d_b2"], dtype=np.float32).copy(),
        "ln1g": np.asarray(inputs["ln1_g"], dtype=np.float32).copy(),
        "ln1b": np.asarray(inputs["ln1_b"], dtype=np.float32).copy(),
        "ln2g": np.asarray(inputs["ln2_g"], dtype=np.float32).copy(),
        "ln2b": np.asarray(inputs["ln2_b"], dtype=np.float32).copy(),
    }

    in_maps = []
    for core in range(NCORES):
        n0 = core * NLOC
        esh = ef[n0:n0 + NLOC]                       # [512, 48, 128]
        ekm = esh.transpose(2, 1, 0)                 # [128c, 48k, 512n]
        ekm = ekm.reshape(C, K, 2, NHALF).transpose(0, 2, 1, 3)  # [c, half, k, n]
        ekm = np.ascontiguousarray(ekm.reshape(C, 2 * K * NHALF))
        msh = mask[n0:n0 + NLOC]
        mask_rm = np.ascontiguousarray(msh.reshape(NRM_T, C).T)  # [128, 4]
        m = dict(shared)
        m["edge_km"] = ekm
        m["x0_ch"] = np.ascontiguousarray(nf[n0:n0 + NLOC].T)
        m["mask_rm"] = mask_rm
        in_maps.append(m)
    return in_maps


def kernel(**inputs) -> np.ndarray:
    if "nc" not in _CACHED:
        _CACHED["nc"] = _build()
    nc = _CACHED["nc"]
    in_maps = _prep_inputs(inputs)
    res = run_bass_kernel_spmd(nc, in_maps, core_ids=list(range(NCORES)))
    out = np.concatenate([res.results[c]["out"] for c in range(NCORES)], axis=0)
    _CACHED["last_results"] = res
    return out



# revision 19
# speedup vs baseline: 1.0603x; 1.0603x over previous
"""Trainium2 Bass kernel for nn_Decoder_24764781429449 (GNN message passing).

Math (per layer l of 3, N=4096 nodes, K=48 neighbors, C=128 channels, H=512):
    base   = concat([node0, zeros, edge])                  # fixed context
    mlp_in = concat([x, base])                             # [N,K,512]
    h1  = gelu(mlp_in @ W1 + b1)
    h2  = gelu(h1 @ W2 + b2)
    msg = h2 @ W3 + b3
    x   = LN1(x + sum_k(msg)/30);  x = LN2(x + dense_mlp(x));  x *= mask

Reductions:
  * W1 rows 256:384 multiply zeros -> dead.
  * x/node0 concat parts are shared across K: h1 = gelu(edge@W1d + pernode),
    pernode = x@W1a + node0@W1b + b1 (node0/b1 parts precomputed on host).
  * sum_k (h2 @ W3 + b3) = PSUM-accumulated per-k W3 matmuls, w3 pre-scaled.
  * LN1 gamma/beta folded into the dense-MLP weights for the dense path.

Distribution: data-parallel over nodes, 512 nodes/core across 8 cores.
Dtypes: edge/h1/h2/message+dense weights bf16 (full PE rate, half DMA);
residual x, pern, LN in fp32/fp32r.
Gelu runs on BOTH the Scalar (ACT table) engine and the Vector engine
(2 custom fused DVE ops: deg-9 odd poly with exact clamp tails, ~1.3e-3 abs
err), with a per-span static engine assignment to balance the two.
The per-node bias (pern) is broadcast over K either by the PE (identity
matmul accumulating into PSUM) or by a DVE scalar_tensor_tensor with a
stride-0 k-repeat, again statically assigned.
LayerNorm tail: PE transposes to row-major, bn_stats from PSUM, ACT sqrt +
DVE reciprocal for rsqrt, normalize on ACT (scale/bias ports), channel
affines in channel-major (per-partition scalars). Tail emission is
interleaved into the next half's span stream so it hides under it.
"""
import os
import numpy as np
import ml_dtypes
from contextlib import ExitStack

import concourse.bass as bass
import concourse.bacc as bacc
import concourse.tile as tile
from concourse import mybir
from concourse.bass_utils import run_bass_kernel_spmd

F32 = mybir.dt.float32
F32R = mybir.dt.float32r
BF16 = mybir.dt.bfloat16
AF = mybir.ActivationFunctionType
OP = mybir.AluOpType
BF = ml_dtypes.bfloat16

N, K, C, E, H, L = 4096, 48, 128, 128, 512, 3
NCORES = 8
NLOC = N // NCORES          # 512 nodes per core
NHALF = NLOC // 2           # 256
KPQ = 4                     # k-values per span
SPAN = KPQ * NHALF          # 1024 columns per span
NSPAN = K // KPQ            # 12 spans per half
SCALE = 30.0
EPS = 1e-5
HCH = H // C                # 4 dense hidden chunks
NRM_T = NLOC // C           # 4 row-major tiles of 128 nodes

# gelu(x) ~= x * clamp(0.5 + x*(GA + GB*u + GC*u^2 + GD*u^3 + GE*u^4), 0, 1)
# u = x^2; max abs err 1.27e-3; tails exact (clamp hits 0/1 by |x|=3.4).
GA, GB, GC, GD, GE = (0.39475106726638376, -0.060296274096807385,
                      0.00686331946932873, -0.00043311219960721437,
                      1.1276622387378065e-05)

_CACHED = {}


# ---------------- custom fused DVE gelu ops ----------------
def _register_gelu_ops():
    if "gelu_ops" in _CACHED:
        return _CACHED["gelu_ops"]
    import concourse.dve_ops as dve_ops
    from concourse.dve_spec import Spec, Src0, Src1, C0, C1, C2, sq, relu, lower
    from concourse.dve_uop import DveOpSpec
    from concourse.dve_table_gen import dve_ver_for

    u = sq(Src0)
    bodyA = (((u * Src1 + C2) * u + C1) * u + C0) * u

    def refA(in0, in1, s0, s1, imm2):
        uu = in0.astype(np.float32) ** 2
        return ((((uu * in1) + imm2) * uu + s1) * uu + s0) * uu

    m = (Src0 + C0) * Src1
    bodyB = (relu(m + C2) - relu(m - C2)) * Src1

    def refB(in0, in1, s0, s1, imm2):
        mm = (in0.astype(np.float32) + s0) * in1
        return (np.maximum(mm + imm2, 0) - np.maximum(mm - imm2, 0)) * in1

    ver = dve_ver_for("TRN2")
    ops = []
    for name, body, ref in (("GELU_P9A_ANT", bodyA, refA),
                            ("GELU_P9B_ANT", bodyB, refB)):
        if name in dve_ops._SUB_OPCODE_FOR_NAME:
            ops.append(next(o for o in dve_ops.OPS if o.name == name))
            continue
        spec = Spec(body=body, reference=ref)
        row = dve_ops._CUSTOM_DVE_ROW_BASE + len(dve_ops.OPS)
        dve_ops._SUB_OPCODE_FOR_NAME[name] = row
        op = dve_ops.DveOp(name, spec, subdim=False, uops_sha={})
        compiled = DveOpSpec(name=name, opcode=row, uops=lower(spec, ver=ver),
                             rd1_en=True)
        object.__setattr__(op, "uops_sha", {ver: compiled.sha(ver)})
        dve_ops.OPS.append(op)
        dve_ops.CUSTOM_DVE_SPECS[name] = spec
        ops.append(op)
    _CACHED["gelu_ops"] = ops
    return ops


# ---------------- packed-constant layout ----------------
class Seg:
    """Column-segment registry for the packed constant tensors."""

    def __init__(self):
        self.cols = 0
        self.off = {}

    def add(self, name, ncols):
        self.off[name] = self.cols
        self.cols += ncols
        return self.off[name]


def _seg_layout():
    bf = Seg()
    for l in range(L):
        bf.add(f"w1d{l}", C)
        bf.add(f"w2{l}", C)
        bf.add(f"w3e{l}", C)
        for ch in range(HCH):
            bf.add(f"dw1_{l}_{ch}", C)
            bf.add(f"dw2_{l}_{ch}", C)
    bf.add("i128bf", C)
    # fp32r segment: everything consumed by fp32r matmuls (PE rounding rule)
    r = Seg()
    r.add("pern0", NLOC)
    r.add("w1a1", C)
    r.add("w1a2", C)
    r.add("i128f", C)
    f = Seg()
    f.add("x0", NLOC)
    f.add("n0pern1", NLOC)
    f.add("n0pern2", NLOC)
    for l in range(L):
        f.add(f"gbc2_{l}", C)
        f.add(f"bbc2_{l}", C)
    f.add("b3e", L)          # [C, l]
    f.add("db2", L)
    f.add("g1", L)
    f.add("b1", L)
    f.add("b2", L)
    f.add("db1", L * HCH)    # [C, (l,ch)]
    f.add("maskrm", NRM_T)
    f.add("eps", 1)
    return bf, f, r


def _build():
    GELU_A, GELU_B = _register_gelu_ops()
    bfseg, fseg, rseg = _seg_layout()

    # ---- per-span engine assignment (env-tunable) ----
    # A-side modes: 0 = PE-pern + ACT gelu, 1 = PE-pern + DVE 2-op gelu,
    #               2 = DVE-stt pern + ACT gelu, 3 = DVE-stt pern + DVE gelu
    # B-side modes: 0 = ACT gelu (bias port), 1 = DVE ts + 2-op gelu
    nA_dve2 = int(os.environ.get("KV_A_DVE2", "2"))
    nA_stta = int(os.environ.get("KV_A_STTA", "3"))
    nA_sttd = int(os.environ.get("KV_A_STTD", "0"))
    nB_dve = int(os.environ.get("KV_B_DVE", "3"))
    V_LAYERS = int(os.environ.get("KV_LAYERS", L))
    NORM_ACT = os.environ.get("KV_NORM_ACT", "1") == "1"

    amodes = [0] * NSPAN
    # spread the special spans across the 12 slots
    special = [1] * nA_dve2 + [2] * nA_stta + [3] * nA_sttd
    if special:
        step = NSPAN / len(special)
        for i, md in enumerate(special):
            amodes[min(NSPAN - 1, int(i * step + step / 2))] = md
    bmodes = [0] * NSPAN
    if nB_dve:
        step = NSPAN / nB_dve
        for i in range(nB_dve):
            bmodes[min(NSPAN - 1, int(i * step))] = 1

    nc = bacc.Bacc()

    edge_d = nc.dram_tensor("edge_km", [C, 2 * K * NHALF], BF16, kind="ExternalInput")
    packbf_d = nc.dram_tensor("packbf", [C, bfseg.cols], BF16, kind="ExternalInput")
    packf_d = nc.dram_tensor("packf", [C, fseg.cols], F32, kind="ExternalInput")
    packr_d = nc.dram_tensor("packr", [C, rseg.cols], F32R, kind="ExternalInput")
    out_d = nc.dram_tensor("out", [NLOC, C], F32, kind="ExternalOutput")

    with tile.TileContext(nc) as tc, ExitStack() as ctx:
        const = ctx.enter_context(tc.tile_pool(name="const", bufs=1))
        h1p = ctx.enter_context(tc.tile_pool(name="h1p", bufs=2))
        h2p = ctx.enter_context(tc.tile_pool(name="h2p", bufs=2))
        xbp = ctx.enter_context(tc.tile_pool(name="xbp", bufs=2))
        tbp = ctx.enter_context(tc.tile_pool(name="tbp", bufs=2))
        tl = ctx.enter_context(tc.tile_pool(name="tl", bufs=2))
        sp = ctx.enter_context(tc.tile_pool(name="sp", bufs=2, space="PSUM"))
        msump = ctx.enter_context(tc.tile_pool(name="msump", bufs=1, space="PSUM"))
        tps = ctx.enter_context(tc.tile_pool(name="tps", bufs=1, space="PSUM"))

        # ---------------- persistent SBUF ----------------
        edge = const.tile([C, 2 * K * NHALF], BF16)
        packbf = const.tile([C, bfseg.cols], BF16)
        packf = const.tile([C, fseg.cols], F32)
        packr = const.tile([C, rseg.cols], F32R)
        ebc = const.tile([C, 1], F32)
        pern = [const.tile([C, NLOC], F32R, name=f"pern{l}") for l in range(1, L)]
        xs = [const.tile([C, NLOC], F32R, name=f"x{l}") for l in range(1, L)]

        def bfv(name, ncols=C, dt=None):
            a = packbf[:, bfseg.off[name]:bfseg.off[name] + ncols]
            return a if dt is None else a.bitcast(dt)

        def fv(name, ncols=1, dt=None):
            a = packf[:, fseg.off[name]:fseg.off[name] + ncols]
            return a if dt is None else a.bitcast(dt)

        def rv(name, ncols=1, dt=None):
            a = packr[:, rseg.off[name]:rseg.off[name] + ncols]
            return a if dt is None else a.bitcast(dt)

        nc.vector.memset(ebc, GE)

        # ---------------- input DMAs ----------------
        nc.sync.dma_start(packf, packf_d.ap())
        nc.sync.dma_start(packr, packr_d.ap())
        nc.sync.dma_start(packbf, packbf_d.ap())
        ECH = 2 * K * NHALF // 8   # 3072-col chunks, 6KB/partition
        for cchunk in range(8):
            sl = slice(cchunk * ECH, (cchunk + 1) * ECH)
            eng = nc.sync if cchunk % 2 == 0 else nc.gpsimd
            eng.dma_start(edge[:, sl], edge_d.ap()[:, sl])

        i128b = bfv("i128bf")
        i128f = rv("i128f", C)
        pern_all = [rv("pern0", NLOC)] + pern
        xs_all = [fv("x0", NLOC, F32R)] + xs
        n0pern = [None, fv("n0pern1", NLOC), fv("n0pern2", NLOC)]
        w1a = [None, rv("w1a1", C), rv("w1a2", C)]
        epsc = fv("eps", 1)

        def vcol(name, l):
            return fv(name, L)[:, l:l + 1]

        # ---------------- DVE gelu helper ----------------
        def emit_gelu_dve(out_ap, x_ap, n, tag):
            t = tbp.tile([C, n], F32, tag="gtb", name=f"gt_{tag}")
            nc.vector._custom_dve(GELU_A, out=t, in0=x_ap,
                                  in1=ebc.broadcast_to([C, n]),
                                  s0=GB, s1=GC, imm2=GD)
            nc.vector._custom_dve(GELU_B, out=out_ap, in0=t, in1=x_ap,
                                  s0=GA, s1=0.0, imm2=0.5)

        # ---------------- stream phase ----------------
        def emit_stream_span(l, h, s, state):
            nsl = slice(h * NHALF, (h + 1) * NHALF)
            amode, bmode = amodes[s], bmodes[s]
            col0 = h * (K * NHALF) + s * SPAN
            lw1d = bfv(f"w1d{l}")
            lw2 = bfv(f"w2{l}")
            lw3e = bfv(f"w3e{l}")

            # -- mm1: edge matmul (+ PE pern broadcast for modes 0/1) --
            t1 = sp.tile([C, SPAN], F32, tag="span", name="t1")
            pe_pern = amode in (0, 1)
            for j in range(2):
                jsl = slice(j * 512, (j + 1) * 512)
                nc.tensor.matmul(t1[:, jsl], lw1d, edge[:, col0 + j * 512:col0 + (j + 1) * 512],
                                 start=True, stop=not pe_pern)
            if pe_pern:
                for q in range(KPQ):
                    rsl = slice(q * NHALF, (q + 1) * NHALF)
                    nc.tensor.matmul(t1[:, rsl], i128f,
                                     pern_all[l][:, nsl], start=False, stop=True)
            state[("t1", s)] = t1

        def emit_gelu_A(l, h, s, state):
            nsl = slice(h * NHALF, (h + 1) * NHALF)
            amode = amodes[s]
            t1 = state.pop(("t1", s))
            state[("t1d", s)] = t1   # dead after gelu-A; reused as t2
            h1 = h1p.tile([C, SPAN], BF16, tag="h1", name="h1")
            if amode in (0, 1):        # pern already in PSUM
                if amode == 0:
                    nc.scalar.activation(h1, t1, AF.Gelu)
                else:
                    emit_gelu_dve(h1, t1, SPAN, f"a{l}{h}{s}")
            else:                      # stt pern-add into SBUF then gelu
                xb = xbp.tile([C, SPAN], F32, tag="xb", name="xb")
                pbc = (pern_all[l].bitcast(F32)[:, nsl]
                       .unsqueeze(1).broadcast_to([C, KPQ, NHALF]))
                nc.vector.scalar_tensor_tensor(
                    xb.rearrange("p (a b) -> p a b", a=KPQ),
                    in0=t1.rearrange("p (a b) -> p a b", a=KPQ),
                    scalar=0.0, in1=pbc, op0=OP.bypass, op1=OP.add)
                if amode == 2:
                    nc.scalar.activation(h1, xb, AF.Gelu)
                else:
                    emit_gelu_dve(h1, xb, SPAN, f"a{l}{h}{s}")
            state[("h1", s)] = h1

        def emit_mm2(l, h, s, state):
            # reuse the span's t1 PSUM tile: gelu-A has consumed it, and the
            # WAR dependency coincides with the RAW dependency on h1.
            h1 = state.pop(("h1", s))
            t2 = state.pop(("t1d", s))
            lw2 = bfv(f"w2{l}")
            for j in range(2):
                jsl = slice(j * 512, (j + 1) * 512)
                nc.tensor.matmul(t2[:, jsl], lw2, h1[:, jsl], start=True, stop=True)
            state[("t2", s)] = t2

        def emit_gelu_B(l, h, s, state):
            t2 = state.pop(("t2", s))
            h2 = h2p.tile([C, SPAN], BF16, tag="h2", name="h2")
            if bmodes[s] == 0:
                nc.scalar.activation(h2, t2, AF.Gelu, bias=vcol("b2", l))
            else:
                xb = xbp.tile([C, SPAN], F32, tag="xb", name="xb2")
                nc.vector.tensor_scalar(xb, t2, vcol("b2", l), None, op0=OP.add)
                emit_gelu_dve(h2, xb, SPAN, f"b{l}{h}{s}")
            state[("h2", s)] = h2

        def emit_msum(l, h, s, state, msum):
            h2 = state.pop(("h2", s))
            lw3e = bfv(f"w3e{l}")
            for q in range(KPQ):
                rsl = slice(q * NHALF, (q + 1) * NHALF)
                nc.tensor.matmul(msum, lw3e, h2[:, rsl],
                                 start=(s == 0 and q == 0),
                                 stop=(s == NSPAN - 1 and q == KPQ - 1))

        # ---------------- tail (generator; pumped between spans) ----------------
        def emit_tail(l, h, msum):
            nsl = slice(h * NHALF, (h + 1) * NHALF)
            last = l == V_LAYERS - 1
            # one 2-bank PSUM tile holds every tail intermediate via regions
            tailt = tps.tile([C, 4 * NHALF], F32, tag="tail", name="tailt")
            x1rm = tailt[:, 0:256].rearrange("p (a b) -> p a b", a=2)
            pd_r = tailt[:, 256:512]
            dd = tailt[:, 512:768]
            xhc_ps = tailt[:, 768:896].bitcast(BF16).rearrange("p (a b) -> p a b", a=2)

            # x1 = x + msum + b3e  (channel-major, fp32)
            x1 = tl.tile([C, NHALF], F32, tag="x1")
            nc.vector.scalar_tensor_tensor(
                x1, in0=msum, scalar=vcol("b3e", l),
                in1=xs_all[l].bitcast(F32)[:, nsl], op0=OP.add, op1=OP.add)
            yield
            # transpose to row-major
            for t in range(2):
                nc.tensor.transpose(x1rm[:, t], x1[:, t * C:(t + 1) * C], i128f.bitcast(F32))
            yield

            def ln_stats(xrm, tag):
                st = tl.tile([C, 2, 6], F32, tag=f"st{tag}")
                mv = tl.tile([C, 2, 2], F32, tag=f"mv{tag}")
                for t in range(2):
                    nc.vector.bn_stats(st[:, t], xrm[:, t])
                for t in range(2):
                    nc.vector.bn_aggr(mv[:, t], st[:, t])
                sd = tl.tile([C, 2], F32, tag=f"sd{tag}")
                var_ap = bass.AP(tensor=mv.tensor, offset=mv.offset + 1,
                                 ap=[list(mv.ap[0])] + [[2, 2]])
                mu_ap = bass.AP(tensor=mv.tensor, offset=mv.offset,
                                ap=[list(mv.ap[0])] + [[2, 2]])
                nc.scalar.activation(sd, var_ap, AF.Sqrt, bias=epsc)
                isd = tl.tile([C, 2], F32, tag=f"isd{tag}")
                nc.vector.reciprocal(isd, sd)
                mui = tl.tile([C, 2], F32, tag=f"mui{tag}")
                nc.vector.scalar_tensor_tensor(mui, in0=mu_ap, scalar=-1.0,
                                               in1=isd, op0=OP.mult, op1=OP.mult)
                return mv, isd, mui

            mv1, isd1, mui1 = ln_stats(x1rm, "1")
            yield
            # normalize -> xhat (rm, bf16); gamma/beta folded into dense wts
            xhat = tl.tile([C, 2, C], BF16, tag="xhat")
            for t in range(2):
                if NORM_ACT:
                    nc.scalar.activation(xhat[:, t], x1rm[:, t], AF.Identity,
                                         bias=mui1[:, t:t + 1], scale=isd1[:, t:t + 1])
                else:
                    nc.vector.tensor_scalar(xhat[:, t], x1rm[:, t],
                                            mv1[:, t, 0:1], isd1[:, t:t + 1],
                                            op0=OP.subtract, op1=OP.mult)
            yield
            # transpose xhat to channel-major
            for t in range(2):
                nc.tensor.transpose(xhc_ps[:, t], xhat[:, t], i128b)
            xhc = tl.tile([C, 2 * C], BF16, tag="xhc")
            nc.vector.tensor_copy(xhc, xhc_ps.rearrange("p a b -> p (a b)"))
            yield
            # dense MLP: 4 chunk rounds; dd accumulates in PSUM
            dh = tl.tile([C, HCH, NHALF], BF16, tag="dh")
            for ch in range(HCH):
                nc.tensor.matmul(pd_r, bfv(f"dw1_{l}_{ch}"), xhc, start=True, stop=True)
                nc.scalar.activation(dh[:, ch], pd_r, AF.Gelu,
                                     bias=fv("db1", L * HCH)[:, l * HCH + ch:l * HCH + ch + 1])
                nc.tensor.matmul(dd, bfv(f"dw2_{l}_{ch}"), dh[:, ch],
                                 start=(ch == 0), stop=(ch == HCH - 1))
                if ch % 2 == 1:
                    yield
            # x2 = (xhat*g1 + b1) + dd + db2   (channel-major)
            x2a = tl.tile([C, NHALF], F32, tag="x2a")
            nc.gpsimd.tensor_scalar(x2a, xhc, vcol("g1", l), vcol("b1", l),
                                    op0=OP.mult, op1=OP.add)
            x2 = tl.tile([C, NHALF], F32, tag="x2")
            nc.vector.scalar_tensor_tensor(x2, in0=dd, scalar=vcol("db2", l),
                                           in1=x2a, op0=OP.add, op1=OP.add)
            yield
            # LN2 (row-major); x1rm region is dead, reuse it
            x2rm = x1rm
            for t in range(2):
                nc.tensor.transpose(x2rm[:, t], x2[:, t * C:(t + 1) * C], i128f.bitcast(F32))
            yield
            mv2, isd2, mui2 = ln_stats(x2rm, "2")
            # fold mask into scale/bias: xhat2 = (x2 - mu)*isd*m
            isdm = tl.tile([C, 2], F32, tag="isdm")
            nc.vector.tensor_mul(isdm, isd2, fv("maskrm", NRM_T)[:, 2 * h:2 * h + 2])
            muim = tl.tile([C, 2], F32, tag="muim")
            nc.vector.tensor_mul(muim, mui2, fv("maskrm", NRM_T)[:, 2 * h:2 * h + 2])
            yield
            xhat2 = tl.tile([C, 2, C], F32, tag="xhat2")
            for t in range(2):
                if NORM_ACT:
                    nc.scalar.activation(xhat2[:, t], x2rm[:, t], AF.Identity,
                                         bias=muim[:, t:t + 1], scale=isdm[:, t:t + 1])
                else:
                    nc.vector.tensor_scalar(xhat2[:, t], x2rm[:, t],
                                            mv2[:, t, 0:1], isdm[:, t:t + 1],
                                            op0=OP.subtract, op1=OP.mult)
            yield
            # x3 = xhat2*gbc2 + bbc2*mask   (row-major)
            x3a = tl.tile([C, 2, C], F32, tag="x3a")
            for t in range(2):
                nc.gpsimd.tensor_mul(x3a[:, t], xhat2[:, t], fv(f"gbc2_{l}", C))
            x3 = tl.tile([C, 2, C], F32, tag="x3")
            for t in range(2):
                nc.vector.scalar_tensor_tensor(
                    x3[:, t], in0=fv(f"bbc2_{l}", C),
                    scalar=fv("maskrm", NRM_T)[:, 2 * h + t:2 * h + t + 1],
                    in1=x3a[:, t], op0=OP.mult, op1=OP.add)
            yield
            if last:
                for t in range(2):
                    nc.sync.dma_start(
                        out_d.ap()[h * NHALF + t * C:h * NHALF + (t + 1) * C, :],
                        x3[:, t])
                return
            # transpose x3 back to channel-major -> xs[l+1]; compute pern[l+1]
            x3c_ps = x2rm  # region free again after the LN2 normalize
            for t in range(2):
                nc.tensor.transpose(x3c_ps[:, t], x3[:, t], i128f.bitcast(F32))
            nc.vector.tensor_copy(xs_all[l + 1][:, nsl],
                                  x3c_ps.rearrange("p a b -> p (a b)"))
            yield
            pp = pd_r  # dense rounds done; reuse that region
            nc.tensor.matmul(pp, w1a[l + 1], xs_all[l + 1][:, nsl], start=True, stop=True)
            nc.vector.scalar_tensor_tensor(
                pern_all[l + 1][:, nsl], in0=pp, scalar=0.0,
                in1=n0pern[l + 1][:, nsl], op0=OP.bypass, op1=OP.add)

        # ---------------- main loop ----------------
        pending_tail = None

        def pump():
            nonlocal pending_tail
            if pending_tail is not None:
                try:
                    next(pending_tail)
                except StopIteration:
                    pending_tail = None

        for l in range(V_LAYERS):
            msumall = msump.tile([C, 2, NHALF], F32, tag="ms", name="msum")
            for h in range(2):
                msum = msumall[:, h]
                state = {}
                for s in range(NSPAN):
                    emit_stream_span(l, h, s, state)
                    if s >= 1:
                        emit_gelu_A(l, h, s - 1, state)
                        emit_mm2(l, h, s - 1, state)
                    if s >= 2:
                        emit_gelu_B(l, h, s - 2, state)
                        emit_msum(l, h, s - 2, state, msum)
                    pump()
                emit_gelu_A(l, h, NSPAN - 1, state)
                emit_mm2(l, h, NSPAN - 1, state)
                for s in (NSPAN - 2, NSPAN - 1):
                    emit_gelu_B(l, h, s, state)
                    emit_msum(l, h, s, state, msum)
                    pump()
                while pending_tail is not None:
                    pump()
                pending_tail = emit_tail(l, h, msum)
        while pending_tail is not None:
            pump()

    nc.compile()
    return nc


def _prep_inputs(inputs):
    """Host-side: shard over nodes, relayout, fold weight-only arithmetic."""
    bfseg, fseg, rseg = _seg_layout()
    nf = np.asarray(inputs["node_features"], dtype=np.float32)
    ef = np.asarray(inputs["edge_features"], dtype=np.float32)
    mask = np.asarray(inputs["mask"], dtype=np.float32)
    w1 = np.asarray(inputs["msg_w1"], dtype=np.float32)
    w2 = np.asarray(inputs["msg_w2"], dtype=np.float32)
    w3 = np.asarray(inputs["msg_w3"], dtype=np.float32)
    b1 = np.asarray(inputs["msg_b1"], dtype=np.float32)
    b2 = np.asarray(inputs["msg_b2"], dtype=np.float32)
    b3 = np.asarray(inputs["msg_b3"], dtype=np.float32)
    dw1 = np.asarray(inputs["d_w1"], dtype=np.float32)
    db1 = np.asarray(inputs["d_b1"], dtype=np.float32)
    dw2 = np.asarray(inputs["d_w2"], dtype=np.float32)
    db2 = np.asarray(inputs["d_b2"], dtype=np.float32)
    g1 = np.asarray(inputs["ln1_g"], dtype=np.float32)
    bb1 = np.asarray(inputs["ln1_b"], dtype=np.float32)
    g2 = np.asarray(inputs["ln2_g"], dtype=np.float32)
    bb2 = np.asarray(inputs["ln2_b"], dtype=np.float32)

    w1a = w1[:, 0:C, :]
    w1b = w1[:, C:2 * C, :]
    w1d = w1[:, 3 * C:4 * C, :]
    w3e = w3 / SCALE
    b3e = b3 * (K / SCALE)
    # dense folds: input is xhat (normalized, no affine); LN1 g/b folded in.
    dw1f = g1[:, :, None] * dw1                     # [L, C, H]
    db1f = db1 + np.einsum("lc,lch->lh", bb1, dw1)  # [L, H]

    # ---- shared packed tensors ----
    packbf = np.zeros((C, bfseg.cols), dtype=BF)
    for l in range(L):
        packbf[:, bfseg.off[f"w1d{l}"]:bfseg.off[f"w1d{l}"] + C] = w1d[l].astype(BF)
        packbf[:, bfseg.off[f"w2{l}"]:bfseg.off[f"w2{l}"] + C] = w2[l].astype(BF)
        packbf[:, bfseg.off[f"w3e{l}"]:bfseg.off[f"w3e{l}"] + C] = w3e[l].astype(BF)
        for ch in range(HCH):
            o = bfseg.off[f"dw1_{l}_{ch}"]
            packbf[:, o:o + C] = dw1f[l][:, ch * C:(ch + 1) * C].astype(BF)
            o = bfseg.off[f"dw2_{l}_{ch}"]
            packbf[:, o:o + C] = dw2[l][ch * C:(ch + 1) * C, :].astype(BF)
    packbf[:, bfseg.off["i128bf"]:bfseg.off["i128bf"] + C] = np.eye(C, dtype=BF)

    packf_shared = np.zeros((C, fseg.cols), dtype=np.float32)
    packr_shared = np.zeros((C, rseg.cols), dtype=np.float32)

    def put(name, arr):
        o = fseg.off[name]
        packf_shared[:, o:o + arr.shape[1]] = arr

    def putr(name, arr):
        o = rseg.off[name]
        packr_shared[:, o:o + arr.shape[1]] = arr

    putr("w1a1", w1a[1]); putr("w1a2", w1a[2])
    putr("i128f", np.eye(C, dtype=np.float32))
    for l in range(L):
        put(f"gbc2_{l}", np.broadcast_to(g2[l][None, :], (C, C)))
        put(f"bbc2_{l}", np.broadcast_to(bb2[l][None, :], (C, C)))
    put("b3e", b3e.T.copy())      # [C, L]
    put("db2", db2.T.copy())
    put("g1", g1.T.copy())
    put("b1", bb1.T.copy())
    put("b2", b2.T.copy())
    put("db1", db1f.reshape(L * HCH, C).T.copy())
    packf_shared[:, fseg.off["eps"]] = EPS

    # host-computed per-node biases
    pern0_full = nf @ (w1a[0] + w1b[0]) + b1[0]          # [N, C]
    n0p1_full = nf @ w1b[1] + b1[1]
    n0p2_full = nf @ w1b[2] + b1[2]

    in_maps = []
    for core in range(NCORES):
        n0 = core * NLOC
        esh = ef[n0:n0 + NLOC]                       # [512, 48, 128]
        ekm = esh.transpose(2, 1, 0)                 # [128c, 48k, 512n]
        ekm = ekm.reshape(C, K, 2, NHALF).transpose(0, 2, 1, 3)
        ekm = np.ascontiguousarray(ekm.reshape(C, 2 * K * NHALF)).astype(BF)
        packf = packf_shared.copy()
        packr = packr_shared.copy()
        o = fseg.off["x0"]
        packf[:, o:o + NLOC] = nf[n0:n0 + NLOC].T
        o = rseg.off["pern0"]
        packr[:, o:o + NLOC] = pern0_full[n0:n0 + NLOC].T
        o = fseg.off["n0pern1"]
        packf[:, o:o + NLOC] = n0p1_full[n0:n0 + NLOC].T
        o = fseg.off["n0pern2"]
        packf[:, o:o + NLOC] = n0p2_full[n0:n0 + NLOC].T
        o = fseg.off["maskrm"]
        packf[:, o:o + NRM_T] = mask[n0:n0 + NLOC].reshape(NRM_T, C).T
        in_maps.append({"edge_km": ekm, "packbf": packbf, "packf": packf,
                        "packr": packr})
    return in_maps


def kernel(**inputs) -> np.ndarray:
    if "nc" not in _CACHED:
        _CACHED["nc"] = _build()
    nc = _CACHED["nc"]
    in_maps = _prep_inputs(inputs)
    res = run_bass_kernel_spmd(nc, in_maps, core_ids=list(range(NCORES)))
    out = np.concatenate([res.results[c]["out"] for c in range(NCORES)], axis=0)
    _CACHED["last_results"] = res
    return out


# revision 23
# speedup vs baseline: 1.1320x; 1.0676x over previous
"""Trainium2 Bass kernel for nn_Decoder_24764781429449 (GNN message passing).

Math (per layer l of 3, N=4096 nodes, K=48 neighbors, C=128 channels, H=512):
    base   = concat([node0, zeros, edge])                  # fixed context
    mlp_in = concat([x, base])                             # [N,K,512]
    h1  = gelu(mlp_in @ W1 + b1)
    h2  = gelu(h1 @ W2 + b2)
    msg = h2 @ W3 + b3
    x   = LN1(x + sum_k(msg)/30);  x = LN2(x + dense_mlp(x));  x *= mask

Reductions:
  * W1 rows 256:384 multiply zeros -> dead.
  * x/node0 concat parts are shared across K: h1 = gelu(edge@W1d + pernode),
    pernode = x@W1a + node0@W1b + b1 (node0/b1 parts precomputed on host).
  * sum_k (h2 @ W3 + b3) = PSUM-accumulated per-k W3 matmuls, w3 pre-scaled.
  * LN1 gamma/beta folded into the dense-MLP weights for the dense path.

Distribution: data-parallel over nodes, 512 nodes/core across 8 cores.
Dtypes: edge/h1/h2/message+dense weights bf16 (full PE rate, half DMA);
residual x, pern, LN in fp32/fp32r.
Gelu runs on BOTH the Scalar (ACT table) engine and the Vector engine
(2 custom fused DVE ops: deg-9 odd poly with exact clamp tails, ~1.3e-3 abs
err), with a per-span static engine assignment to balance the two.
The per-node bias (pern) is broadcast over K either by the PE (identity
matmul accumulating into PSUM) or by a DVE scalar_tensor_tensor with a
stride-0 k-repeat, again statically assigned.
LayerNorm tail: PE transposes to row-major, bn_stats from PSUM, ACT sqrt +
DVE reciprocal for rsqrt, normalize on ACT (scale/bias ports), channel
affines in channel-major (per-partition scalars). Tail emission is
interleaved into the next half's span stream so it hides under it.
"""
import os
import numpy as np
import ml_dtypes
from contextlib import ExitStack

import concourse.bass as bass
import concourse.bacc as bacc
import concourse.tile as tile
from concourse import mybir
from concourse.bass_utils import run_bass_kernel_spmd

F32 = mybir.dt.float32
F32R = mybir.dt.float32r
BF16 = mybir.dt.bfloat16
AF = mybir.ActivationFunctionType
OP = mybir.AluOpType
BF = ml_dtypes.bfloat16

N, K, C, E, H, L = 4096, 48, 128, 128, 512, 3
NCORES = 8
NLOC = N // NCORES          # 512 nodes per core
NHALF = NLOC // 2           # 256
KPQ = 4                     # k-values per span
SPAN = KPQ * NHALF          # 1024 columns per span
NSPAN = K // KPQ            # 12 spans per half
SCALE = 30.0
EPS = 1e-5
HCH = H // C                # 4 dense hidden chunks
NRM_T = NLOC // C           # 4 row-major tiles of 128 nodes

# gelu(x) ~= x * clamp(0.5 + x*(GA + GB*u + GC*u^2 + GD*u^3 + GE*u^4), 0, 1)
# u = x^2; max abs err 1.27e-3; tails exact (clamp hits 0/1 by |x|=3.4).
GA, GB, GC, GD, GE = (0.39475106726638376, -0.060296274096807385,
                      0.00686331946932873, -0.00043311219960721437,
                      1.1276622387378065e-05)

_CACHED = {}


# ---------------- custom fused DVE gelu ops ----------------
def _register_gelu_ops():
    if "gelu_ops" in _CACHED:
        return _CACHED["gelu_ops"]
    import concourse.dve_ops as dve_ops
    from concourse.dve_spec import Spec, Src0, Src1, C0, C1, C2, sq, relu, lower
    from concourse.dve_uop import DveOpSpec
    from concourse.dve_table_gen import dve_ver_for

    u = sq(Src0)
    bodyA = (((u * Src1 + C2) * u + C1) * u + C0) * u

    def refA(in0, in1, s0, s1, imm2):
        uu = in0.astype(np.float32) ** 2
        return ((((uu * in1) + imm2) * uu + s1) * uu + s0) * uu

    m = (Src0 + C0) * Src1
    bodyB = (relu(m + C2) - relu(m - C2)) * Src1

    def refB(in0, in1, s0, s1, imm2):
        mm = (in0.astype(np.float32) + s0) * in1
        return (np.maximum(mm + imm2, 0) - np.maximum(mm - imm2, 0)) * in1

    ver = dve_ver_for("TRN2")
    ops = []
    for name, body, ref in (("GELU_P9A_ANT", bodyA, refA),
                            ("GELU_P9B_ANT", bodyB, refB)):
        if name in dve_ops._SUB_OPCODE_FOR_NAME:
            ops.append(next(o for o in dve_ops.OPS if o.name == name))
            continue
        spec = Spec(body=body, reference=ref)
        row = dve_ops._CUSTOM_DVE_ROW_BASE + len(dve_ops.OPS)
        dve_ops._SUB_OPCODE_FOR_NAME[name] = row
        op = dve_ops.DveOp(name, spec, subdim=False, uops_sha={})
        compiled = DveOpSpec(name=name, opcode=row, uops=lower(spec, ver=ver),
                             rd1_en=True)
        object.__setattr__(op, "uops_sha", {ver: compiled.sha(ver)})
        dve_ops.OPS.append(op)
        dve_ops.CUSTOM_DVE_SPECS[name] = spec
        ops.append(op)
    _CACHED["gelu_ops"] = ops
    return ops


# ---------------- packed-constant layout ----------------
class Seg:
    """Column-segment registry for the packed constant tensors."""

    def __init__(self):
        self.cols = 0
        self.off = {}

    def add(self, name, ncols):
        self.off[name] = self.cols
        self.cols += ncols
        return self.off[name]


def _seg_layout():
    bf = Seg()
    for l in range(L):
        bf.add(f"w1d{l}", C)
        bf.add(f"w2{l}", C)
        bf.add(f"w3e{l}", C)
        for ch in range(HCH):
            bf.add(f"dw1_{l}_{ch}", C)
            bf.add(f"dw2_{l}_{ch}", C)
    bf.add("i128bf", C)
    # fp32r segment: everything consumed by fp32r matmuls (PE rounding rule)
    r = Seg()
    r.add("pern0", NLOC)
    r.add("w1a1", C)
    r.add("w1a2", C)
    r.add("i128f", C)
    f = Seg()
    f.add("x0", NLOC)
    f.add("n0pern1", NLOC)
    f.add("n0pern2", NLOC)
    for l in range(L):
        f.add(f"gbc2_{l}", C)
        f.add(f"bbc2_{l}", C)
    f.add("b3e", L)          # [C, l]
    f.add("db2", L)
    f.add("g1", L)
    f.add("b1", L)
    f.add("b2", L)
    f.add("db1", L * HCH)    # [C, (l,ch)]
    f.add("maskrm", NRM_T)
    f.add("eps", 1)
    return bf, f, r


def _build():
    GELU_A, GELU_B = _register_gelu_ops()
    bfseg, fseg, rseg = _seg_layout()

    # ---- per-span engine assignment (env-tunable) ----
    # A-side modes: 0 = PE-pern + ACT gelu, 1 = PE-pern + DVE 2-op gelu,
    #               2 = DVE-stt pern + ACT gelu, 3 = DVE-stt pern + DVE gelu
    # B-side modes: 0 = ACT gelu (bias port), 1 = DVE ts + 2-op gelu
    nA_dve2 = int(os.environ.get("KV_A_DVE2", "2"))
    nA_stta = int(os.environ.get("KV_A_STTA", "3"))
    nA_sttd = int(os.environ.get("KV_A_STTD", "0"))
    nB_dve = int(os.environ.get("KV_B_DVE", "3"))
    V_LAYERS = int(os.environ.get("KV_LAYERS", L))
    NORM_ACT = os.environ.get("KV_NORM_ACT", "1") == "1"

    amodes = [0] * NSPAN
    # spread the special spans across the 12 slots
    special = [1] * nA_dve2 + [2] * nA_stta + [3] * nA_sttd
    if special:
        step = NSPAN / len(special)
        for i, md in enumerate(special):
            amodes[min(NSPAN - 1, int(i * step + step / 2))] = md
    bmodes = [0] * NSPAN
    if nB_dve:
        step = NSPAN / nB_dve
        for i in range(nB_dve):
            bmodes[min(NSPAN - 1, int(i * step))] = 1

    nc = bacc.Bacc()

    edge_d = nc.dram_tensor("edge_km", [C, 2 * K * NHALF], BF16, kind="ExternalInput")
    packbf_d = nc.dram_tensor("packbf", [C, bfseg.cols], BF16, kind="ExternalInput")
    packf_d = nc.dram_tensor("packf", [C, fseg.cols], F32, kind="ExternalInput")
    packr_d = nc.dram_tensor("packr", [C, rseg.cols], F32R, kind="ExternalInput")
    out_d = nc.dram_tensor("out", [NLOC, C], F32, kind="ExternalOutput")

    with tile.TileContext(nc) as tc, ExitStack() as ctx:
        const = ctx.enter_context(tc.tile_pool(name="const", bufs=1))
        h1p = ctx.enter_context(tc.tile_pool(name="h1p", bufs=2))
        h2p = ctx.enter_context(tc.tile_pool(name="h2p", bufs=2))
        xbp = ctx.enter_context(tc.tile_pool(name="xbp", bufs=2))
        tbp = ctx.enter_context(tc.tile_pool(name="tbp", bufs=2))
        tl = ctx.enter_context(tc.tile_pool(name="tl", bufs=2))
        sp = ctx.enter_context(tc.tile_pool(name="sp", bufs=2, space="PSUM"))
        msump = ctx.enter_context(tc.tile_pool(name="msump", bufs=1, space="PSUM"))
        tps = ctx.enter_context(tc.tile_pool(name="tps", bufs=1, space="PSUM"))

        # ---------------- persistent SBUF ----------------
        edge = const.tile([C, 2 * K * NHALF], BF16)
        packbf = const.tile([C, bfseg.cols], BF16)
        packf = const.tile([C, fseg.cols], F32)
        packr = const.tile([C, rseg.cols], F32R)
        ebc = const.tile([C, 1], F32)
        magic = const.tile([C, 1], mybir.dt.int32)
        pern = [const.tile([C, NLOC], F32R, name=f"pern{l}") for l in range(1, L)]
        xs = [const.tile([C, NLOC], F32R, name=f"x{l}") for l in range(1, L)]

        def bfv(name, ncols=C, dt=None):
            a = packbf[:, bfseg.off[name]:bfseg.off[name] + ncols]
            return a if dt is None else a.bitcast(dt)

        def fv(name, ncols=1, dt=None):
            a = packf[:, fseg.off[name]:fseg.off[name] + ncols]
            return a if dt is None else a.bitcast(dt)

        def rv(name, ncols=1, dt=None):
            a = packr[:, rseg.off[name]:rseg.off[name] + ncols]
            return a if dt is None else a.bitcast(dt)

        nc.vector.memset(ebc, GE)
        nc.vector.memset(magic, 0x5F3759DF)

        # ---------------- input DMAs ----------------
        # order: small fp32r pack (pern0/w1a/i128) + layer-0 weights slice of
        # the bf16 pack, then edge in span-sized chunks for the first spans so
        # layer-0 compute starts ~7us in, bulk packs late on the gpsimd queue.
        nc.sync.dma_start(packr, packr_d.ap())
        l0cols = bfseg.off["w3e0"] + C   # w1d0/w2_0/w3e0 prefix
        nc.sync.dma_start(packbf[:, 0:l0cols], packbf_d.ap()[:, 0:l0cols])
        nc.gpsimd.dma_start(packf, packf_d.ap())
        edge_chunks = [SPAN, SPAN, 2 * SPAN, 4 * SPAN, 8 * SPAN, 8 * SPAN]
        c0 = 0
        for i, w in enumerate(edge_chunks):
            sl = slice(c0, c0 + w)
            eng = nc.sync if i % 2 == 0 or i < 3 else nc.gpsimd
            eng.dma_start(edge[:, sl], edge_d.ap()[:, sl])
            c0 += w
        assert c0 == 2 * K * NHALF, c0
        nc.gpsimd.dma_start(packbf[:, l0cols:], packbf_d.ap()[:, l0cols:])

        i128b = bfv("i128bf")
        i128f = rv("i128f", C)
        pern_all = [rv("pern0", NLOC)] + pern
        xs_all = [fv("x0", NLOC, F32R)] + xs
        n0pern = [None, fv("n0pern1", NLOC), fv("n0pern2", NLOC)]
        w1a = [None, rv("w1a1", C), rv("w1a2", C)]
        epsc = fv("eps", 1)

        def vcol(name, l):
            return fv(name, L)[:, l:l + 1]

        # ---------------- DVE gelu helper ----------------
        def emit_gelu_dve(out_ap, x_ap, n, tag):
            t = tbp.tile([C, n], F32, tag="gtb", name=f"gt_{tag}")
            nc.vector._custom_dve(GELU_A, out=t, in0=x_ap,
                                  in1=ebc.broadcast_to([C, n]),
                                  s0=GB, s1=GC, imm2=GD)
            nc.vector._custom_dve(GELU_B, out=out_ap, in0=t, in1=x_ap,
                                  s0=GA, s1=0.0, imm2=0.5)

        # ---------------- stream phase ----------------
        def emit_stream_span(l, h, s, state):
            nsl = slice(h * NHALF, (h + 1) * NHALF)
            amode, bmode = amodes[s], bmodes[s]
            col0 = h * (K * NHALF) + s * SPAN
            lw1d = bfv(f"w1d{l}")
            lw2 = bfv(f"w2{l}")
            lw3e = bfv(f"w3e{l}")

            # -- mm1: edge matmul (+ PE pern broadcast for modes 0/1) --
            t1 = sp.tile([C, SPAN], F32, tag="span", name="t1")
            pe_pern = amode in (0, 1)
            for j in range(2):
                jsl = slice(j * 512, (j + 1) * 512)
                nc.tensor.matmul(t1[:, jsl], lw1d, edge[:, col0 + j * 512:col0 + (j + 1) * 512],
                                 start=True, stop=not pe_pern)
            if pe_pern:
                for q in range(KPQ):
                    rsl = slice(q * NHALF, (q + 1) * NHALF)
                    nc.tensor.matmul(t1[:, rsl], i128f,
                                     pern_all[l][:, nsl], start=False, stop=True)
            state[("t1", s)] = t1

        def emit_gelu_A(l, h, s, state):
            nsl = slice(h * NHALF, (h + 1) * NHALF)
            amode = amodes[s]
            t1 = state.pop(("t1", s))
            state[("t1d", s)] = t1   # dead after gelu-A; reused as t2
            h1 = h1p.tile([C, SPAN], BF16, tag="h1", name="h1")
            if amode in (0, 1):        # pern already in PSUM
                if amode == 0:
                    nc.scalar.activation(h1, t1, AF.Gelu)
                else:
                    emit_gelu_dve(h1, t1, SPAN, f"a{l}{h}{s}")
            else:                      # stt pern-add into SBUF then gelu
                xb = xbp.tile([C, SPAN], F32, tag="xb", name="xb")
                pbc = (pern_all[l].bitcast(F32)[:, nsl]
                       .unsqueeze(1).broadcast_to([C, KPQ, NHALF]))
                nc.vector.scalar_tensor_tensor(
                    xb.rearrange("p (a b) -> p a b", a=KPQ),
                    in0=t1.rearrange("p (a b) -> p a b", a=KPQ),
                    scalar=0.0, in1=pbc, op0=OP.bypass, op1=OP.add)
                if amode == 2:
                    nc.scalar.activation(h1, xb, AF.Gelu)
                else:
                    emit_gelu_dve(h1, xb, SPAN, f"a{l}{h}{s}")
            state[("h1", s)] = h1

        def emit_mm2(l, h, s, state):
            # reuse the span's t1 PSUM tile: gelu-A has consumed it, and the
            # WAR dependency coincides with the RAW dependency on h1.
            h1 = state.pop(("h1", s))
            t2 = state.pop(("t1d", s))
            lw2 = bfv(f"w2{l}")
            for j in range(2):
                jsl = slice(j * 512, (j + 1) * 512)
                nc.tensor.matmul(t2[:, jsl], lw2, h1[:, jsl], start=True, stop=True)
            state[("t2", s)] = t2

        def emit_gelu_B(l, h, s, state):
            t2 = state.pop(("t2", s))
            h2 = h2p.tile([C, SPAN], BF16, tag="h2", name="h2")
            if bmodes[s] == 0:
                nc.scalar.activation(h2, t2, AF.Gelu, bias=vcol("b2", l))
            else:
                xb = xbp.tile([C, SPAN], F32, tag="xb", name="xb2")
                nc.vector.tensor_scalar(xb, t2, vcol("b2", l), None, op0=OP.add)
                emit_gelu_dve(h2, xb, SPAN, f"b{l}{h}{s}")
            state[("h2", s)] = h2

        def emit_msum(l, h, s, state, msum):
            h2 = state.pop(("h2", s))
            lw3e = bfv(f"w3e{l}")
            for q in range(KPQ):
                rsl = slice(q * NHALF, (q + 1) * NHALF)
                nc.tensor.matmul(msum, lw3e, h2[:, rsl],
                                 start=(s == 0 and q == 0),
                                 stop=(s == NSPAN - 1 and q == KPQ - 1))

        # ---------------- tail (generator; pumped between spans) ----------------
        def emit_tail(l, h, msum):
            nsl = slice(h * NHALF, (h + 1) * NHALF)
            last = l == V_LAYERS - 1
            # one 2-bank PSUM tile holds every tail intermediate via regions
            tailt = tps.tile([C, 4 * NHALF], F32, tag="tail", name="tailt")
            x1rm = tailt[:, 0:256].rearrange("p (a b) -> p a b", a=2)
            pd_r = tailt[:, 256:512]
            dd = tailt[:, 512:768]
            xhc_ps = tailt[:, 768:896].bitcast(BF16).rearrange("p (a b) -> p a b", a=2)

            # x1 = x + msum + b3e  (channel-major, fp32)
            x1 = tl.tile([C, NHALF], F32, tag="x1")
            nc.vector.scalar_tensor_tensor(
                x1, in0=msum, scalar=vcol("b3e", l),
                in1=xs_all[l].bitcast(F32)[:, nsl], op0=OP.add, op1=OP.add)
            yield
            # transpose to row-major
            for t in range(2):
                nc.tensor.transpose(x1rm[:, t], x1[:, t * C:(t + 1) * C], i128f.bitcast(F32))
            yield

            def ln_stats(xrm, tag):
                # stats on DVE; rsqrt + helpers on the (otherwise idle) Pool
                # engine via the quake bit-hack -- keeps the ACT engine on the
                # gelu table (a Sqrt activation would force 1.3us table swaps).
                st = tl.tile([C, 2, 6], F32, tag=f"st{tag}")
                mv = tl.tile([C, 2, 2], F32, tag=f"mv{tag}")
                for t in range(2):
                    nc.vector.bn_stats(st[:, t], xrm[:, t])
                for t in range(2):
                    nc.vector.bn_aggr(mv[:, t], st[:, t])
                var_ap = bass.AP(tensor=mv.tensor, offset=mv.offset + 1,
                                 ap=[list(mv.ap[0])] + [[2, 2]])
                mu_ap = bass.AP(tensor=mv.tensor, offset=mv.offset,
                                ap=[list(mv.ap[0])] + [[2, 2]])
                veps = tl.tile([C, 2], F32, tag=f"ve{tag}")
                nc.gpsimd.tensor_scalar(veps, var_ap, EPS, None, op0=OP.add)
                isd = tl.tile([C, 2], F32, tag=f"isd{tag}")
                ush = tl.tile([C, 2], mybir.dt.int32, tag=f"us{tag}")
                nc.vector.tensor_scalar(ush, veps.bitcast(mybir.dt.int32), 1, None,
                                        op0=OP.logical_shift_right)
                nc.vector.scalar_tensor_tensor(
                    isd.bitcast(mybir.dt.int32), in0=magic.broadcast_to([C, 2]),
                    scalar=0, in1=ush, op0=OP.bypass, op1=OP.subtract)
                qt = tl.tile([C, 2], F32, tag=f"qt{tag}")
                for _ in range(2):
                    nc.gpsimd.tensor_mul(qt, isd, isd)
                    nc.gpsimd.tensor_mul(qt, qt, veps)
                    nc.gpsimd.tensor_scalar(qt, qt, -0.5, 1.5, op0=OP.mult, op1=OP.add)
                    nc.gpsimd.tensor_mul(isd, isd, qt)
                mui = tl.tile([C, 2], F32, tag=f"mui{tag}")
                nc.vector.scalar_tensor_tensor(mui, in0=mu_ap, scalar=-1.0,
                                               in1=isd, op0=OP.mult, op1=OP.mult)
                return mv, isd, mui

            mv1, isd1, mui1 = ln_stats(x1rm, "1")
            yield
            # normalize -> xhat (rm, bf16); gamma/beta folded into dense wts
            xhat = tl.tile([C, 2, C], BF16, tag="xhat")
            for t in range(2):
                if NORM_ACT:
                    nc.scalar.activation(xhat[:, t], x1rm[:, t], AF.Identity,
                                         bias=mui1[:, t:t + 1], scale=isd1[:, t:t + 1])
                else:
                    nc.vector.tensor_scalar(xhat[:, t], x1rm[:, t],
                                            mv1[:, t, 0:1], isd1[:, t:t + 1],
                                            op0=OP.subtract, op1=OP.mult)
            yield
            # transpose xhat to channel-major
            for t in range(2):
                nc.tensor.transpose(xhc_ps[:, t], xhat[:, t], i128b)
            xhc = tl.tile([C, 2 * C], BF16, tag="xhc")
            nc.vector.tensor_copy(xhc, xhc_ps.rearrange("p a b -> p (a b)"))
            yield
            # dense MLP: 4 chunk rounds; dd accumulates in PSUM
            dh = tl.tile([C, HCH, NHALF], BF16, tag="dh")
            for ch in range(HCH):
                nc.tensor.matmul(pd_r, bfv(f"dw1_{l}_{ch}"), xhc, start=True, stop=True)
                nc.scalar.activation(dh[:, ch], pd_r, AF.Gelu,
                                     bias=fv("db1", L * HCH)[:, l * HCH + ch:l * HCH + ch + 1])
                nc.tensor.matmul(dd, bfv(f"dw2_{l}_{ch}"), dh[:, ch],
                                 start=(ch == 0), stop=(ch == HCH - 1))
                if ch % 2 == 1:
                    yield
            # x2 = (xhat*g1 + b1) + dd + db2   (channel-major)
            x2a = tl.tile([C, NHALF], F32, tag="x2a")
            nc.gpsimd.tensor_mul(x2a, xhc, vcol("g1", l).broadcast_to([C, NHALF]))
            nc.gpsimd.tensor_add(x2a, x2a, vcol("b1", l).broadcast_to([C, NHALF]))
            x2 = tl.tile([C, NHALF], F32, tag="x2")
            nc.vector.scalar_tensor_tensor(x2, in0=dd, scalar=vcol("db2", l),
                                           in1=x2a, op0=OP.add, op1=OP.add)
            yield
            # LN2 (row-major); x1rm region is dead, reuse it
            x2rm = x1rm
            for t in range(2):
                nc.tensor.transpose(x2rm[:, t], x2[:, t * C:(t + 1) * C], i128f.bitcast(F32))
            yield
            mv2, isd2, mui2 = ln_stats(x2rm, "2")
            # fold mask into scale/bias: xhat2 = (x2 - mu)*isd*m
            isdm = tl.tile([C, 2], F32, tag="isdm")
            nc.gpsimd.tensor_mul(isdm, isd2, fv("maskrm", NRM_T)[:, 2 * h:2 * h + 2])
            muim = tl.tile([C, 2], F32, tag="muim")
            nc.gpsimd.tensor_mul(muim, mui2, fv("maskrm", NRM_T)[:, 2 * h:2 * h + 2])
            yield
            xhat2 = tl.tile([C, 2, C], F32, tag="xhat2")
            for t in range(2):
                if NORM_ACT:
                    nc.scalar.activation(xhat2[:, t], x2rm[:, t], AF.Identity,
                                         bias=muim[:, t:t + 1], scale=isdm[:, t:t + 1])
                else:
                    nc.vector.tensor_scalar(xhat2[:, t], x2rm[:, t],
                                            mv2[:, t, 0:1], isdm[:, t:t + 1],
                                            op0=OP.subtract, op1=OP.mult)
            yield
            # x3 = xhat2*gbc2 + bbc2*mask   (row-major)
            x3a = tl.tile([C, 2, C], F32, tag="x3a")
            for t in range(2):
                nc.gpsimd.tensor_mul(x3a[:, t], xhat2[:, t], fv(f"gbc2_{l}", C))
            x3 = tl.tile([C, 2, C], F32, tag="x3")
            for t in range(2):
                nc.vector.scalar_tensor_tensor(
                    x3[:, t], in0=fv(f"bbc2_{l}", C),
                    scalar=fv("maskrm", NRM_T)[:, 2 * h + t:2 * h + t + 1],
                    in1=x3a[:, t], op0=OP.mult, op1=OP.add)
            yield
            if last:
                for t in range(2):
                    nc.sync.dma_start(
                        out_d.ap()[h * NHALF + t * C:h * NHALF + (t + 1) * C, :],
                        x3[:, t])
                return
            # transpose x3 back to channel-major -> xs[l+1]; compute pern[l+1]
            x3c_ps = x2rm  # region free again after the LN2 normalize
            for t in range(2):
                nc.tensor.transpose(x3c_ps[:, t], x3[:, t], i128f.bitcast(F32))
            nc.vector.tensor_copy(xs_all[l + 1][:, nsl],
                                  x3c_ps.rearrange("p a b -> p (a b)"))
            yield
            pp = pd_r  # dense rounds done; reuse that region
            nc.tensor.matmul(pp, w1a[l + 1], xs_all[l + 1][:, nsl], start=True, stop=True)
            nc.vector.scalar_tensor_tensor(
                pern_all[l + 1][:, nsl], in0=pp, scalar=0.0,
                in1=n0pern[l + 1][:, nsl], op0=OP.bypass, op1=OP.add)

        # ---------------- main loop ----------------
        pending_tail = None

        def pump():
            nonlocal pending_tail
            if pending_tail is not None:
                try:
                    next(pending_tail)
                except StopIteration:
                    pending_tail = None

        for l in range(V_LAYERS):
            msumall = msump.tile([C, 2, NHALF], F32, tag="ms", name="msum")
            for h in range(2):
                msum = msumall[:, h]
                state = {}
                for s in range(NSPAN):
                    emit_stream_span(l, h, s, state)
                    if s >= 1:
                        emit_gelu_A(l, h, s - 1, state)
                        emit_mm2(l, h, s - 1, state)
                    if s >= 2:
                        emit_gelu_B(l, h, s - 2, state)
                        emit_msum(l, h, s - 2, state, msum)
                    pump()
                emit_gelu_A(l, h, NSPAN - 1, state)
                emit_mm2(l, h, NSPAN - 1, state)
                for s in (NSPAN - 2, NSPAN - 1):
                    emit_gelu_B(l, h, s, state)
                    emit_msum(l, h, s, state, msum)
                    pump()
                while pending_tail is not None:
                    pump()
                pending_tail = emit_tail(l, h, msum)
        while pending_tail is not None:
            pump()

    nc.compile()
    return nc


def _prep_inputs(inputs):
    """Host-side: shard over nodes, relayout, fold weight-only arithmetic."""
    bfseg, fseg, rseg = _seg_layout()
    nf = np.asarray(inputs["node_features"], dtype=np.float32)
    ef = np.asarray(inputs["edge_features"], dtype=np.float32)
    mask = np.asarray(inputs["mask"], dtype=np.float32)
    w1 = np.asarray(inputs["msg_w1"], dtype=np.float32)
    w2 = np.asarray(inputs["msg_w2"], dtype=np.float32)
    w3 = np.asarray(inputs["msg_w3"], dtype=np.float32)
    b1 = np.asarray(inputs["msg_b1"], dtype=np.float32)
    b2 = np.asarray(inputs["msg_b2"], dtype=np.float32)
    b3 = np.asarray(inputs["msg_b3"], dtype=np.float32)
    dw1 = np.asarray(inputs["d_w1"], dtype=np.float32)
    db1 = np.asarray(inputs["d_b1"], dtype=np.float32)
    dw2 = np.asarray(inputs["d_w2"], dtype=np.float32)
    db2 = np.asarray(inputs["d_b2"], dtype=np.float32)
    g1 = np.asarray(inputs["ln1_g"], dtype=np.float32)
    bb1 = np.asarray(inputs["ln1_b"], dtype=np.float32)
    g2 = np.asarray(inputs["ln2_g"], dtype=np.float32)
    bb2 = np.asarray(inputs["ln2_b"], dtype=np.float32)

    w1a = w1[:, 0:C, :]
    w1b = w1[:, C:2 * C, :]
    w1d = w1[:, 3 * C:4 * C, :]
    w3e = w3 / SCALE
    b3e = b3 * (K / SCALE)
    # dense folds: input is xhat (normalized, no affine); LN1 g/b folded in.
    dw1f = g1[:, :, None] * dw1                     # [L, C, H]
    db1f = db1 + np.einsum("lc,lch->lh", bb1, dw1)  # [L, H]

    # ---- shared packed tensors ----
    packbf = np.zeros((C, bfseg.cols), dtype=BF)
    for l in range(L):
        packbf[:, bfseg.off[f"w1d{l}"]:bfseg.off[f"w1d{l}"] + C] = w1d[l].astype(BF)
        packbf[:, bfseg.off[f"w2{l}"]:bfseg.off[f"w2{l}"] + C] = w2[l].astype(BF)
        packbf[:, bfseg.off[f"w3e{l}"]:bfseg.off[f"w3e{l}"] + C] = w3e[l].astype(BF)
        for ch in range(HCH):
            o = bfseg.off[f"dw1_{l}_{ch}"]
            packbf[:, o:o + C] = dw1f[l][:, ch * C:(ch + 1) * C].astype(BF)
            o = bfseg.off[f"dw2_{l}_{ch}"]
            packbf[:, o:o + C] = dw2[l][ch * C:(ch + 1) * C, :].astype(BF)
    packbf[:, bfseg.off["i128bf"]:bfseg.off["i128bf"] + C] = np.eye(C, dtype=BF)

    packf_shared = np.zeros((C, fseg.cols), dtype=np.float32)
    packr_shared = np.zeros((C, rseg.cols), dtype=np.float32)

    def put(name, arr):
        o = fseg.off[name]
        packf_shared[:, o:o + arr.shape[1]] = arr

    def putr(name, arr):
        o = rseg.off[name]
        packr_shared[:, o:o + arr.shape[1]] = arr

    putr("w1a1", w1a[1]); putr("w1a2", w1a[2])
    putr("i128f", np.eye(C, dtype=np.float32))
    for l in range(L):
        put(f"gbc2_{l}", np.broadcast_to(g2[l][None, :], (C, C)))
        put(f"bbc2_{l}", np.broadcast_to(bb2[l][None, :], (C, C)))
    put("b3e", b3e.T.copy())      # [C, L]
    put("db2", db2.T.copy())
    put("g1", g1.T.copy())
    put("b1", bb1.T.copy())
    put("b2", b2.T.copy())
    put("db1", db1f.reshape(L * HCH, C).T.copy())
    packf_shared[:, fseg.off["eps"]] = EPS

    # host-computed per-node biases
    pern0_full = nf @ (w1a[0] + w1b[0]) + b1[0]          # [N, C]
    n0p1_full = nf @ w1b[1] + b1[1]
    n0p2_full = nf @ w1b[2] + b1[2]

    in_maps = []
    for core in range(NCORES):
        n0 = core * NLOC
        esh = ef[n0:n0 + NLOC]                       # [512, 48, 128]
        ekm = esh.transpose(2, 1, 0)                 # [128c, 48k, 512n]
        ekm = ekm.reshape(C, K, 2, NHALF).transpose(0, 2, 1, 3)
        ekm = np.ascontiguousarray(ekm.reshape(C, 2 * K * NHALF)).astype(BF)
        packf = packf_shared.copy()
        packr = packr_shared.copy()
        o = fseg.off["x0"]
        packf[:, o:o + NLOC] = nf[n0:n0 + NLOC].T
        o = rseg.off["pern0"]
        packr[:, o:o + NLOC] = pern0_full[n0:n0 + NLOC].T
        o = fseg.off["n0pern1"]
        packf[:, o:o + NLOC] = n0p1_full[n0:n0 + NLOC].T
        o = fseg.off["n0pern2"]
        packf[:, o:o + NLOC] = n0p2_full[n0:n0 + NLOC].T
        o = fseg.off["maskrm"]
        packf[:, o:o + NRM_T] = mask[n0:n0 + NLOC].reshape(NRM_T, C).T
        in_maps.append({"edge_km": ekm, "packbf": packbf, "packf": packf,
                        "packr": packr})
    return in_maps


def kernel(**inputs) -> np.ndarray:
    if "nc" not in _CACHED:
        _CACHED["nc"] = _build()
    nc = _CACHED["nc"]
    in_maps = _prep_inputs(inputs)
    res = run_bass_kernel_spmd(nc, in_maps, core_ids=list(range(NCORES)))
    out = np.concatenate([res.results[c]["out"] for c in range(NCORES)], axis=0)
    _CACHED["last_results"] = res
    return out
